# revision 4
# baseline (speedup 1.0000x reference)
"""Deformable attention TRN2 kernel v2 (quad-token ap_gather).

Changes vs v1: one d=4 gather fetches all 4 bilinear corners (halves gather
count); bilinear weights + gather indices precomputed on host; v-wave weight
replication via PE outer-product instead of partition-broadcast DMA.
Core 2*b+half computes output rows [24*half, 24*half+24) of batch b.
"""
import sys
from contextlib import ExitStack

import numpy as np

if "/opt/trn_rl_repo" not in sys.path:
    sys.path.insert(0, "/opt/trn_rl_repo")

import ml_dtypes
import concourse.bass as bass
import concourse.tile as tile
from concourse import mybir
from concourse._compat import with_exitstack

AF = mybir.ActivationFunctionType
ALU = mybir.AluOpType
DT = mybir.dt

H = W = 48
HP = 24
N = HP * W              # 1152 pixels per core
C = 192
HEADS, D = 12, 16
DG, K, CLIP = 12, 9, 2
M3 = 49 * 49            # 2401 quad-anchor tokens (row 48 = zero pad)
SCALE = float(D) ** -0.5

TILE_SLOTS = [
    [(0, h) for h in range(8)],
    [(1, h) for h in range(8)],
    [(0, 8), (0, 9), (0, 10), (0, 11), (1, 8), (1, 9), (1, 10), (1, 11)],
]
KY = np.repeat(np.arange(3), 3).astype(np.float32)
KX = np.tile(np.arange(3), 3).astype(np.float32)

F32, BF16, I16 = DT.float32, DT.bfloat16, DT.int16
BF = ml_dtypes.bfloat16


# ======================================================================
# host prep
# ======================================================================

def _quad(off, n_row, n_col, c, g, kk):
    """Anchor token + 4 corner weights for (clip c, group g, tap kk)."""
    py = off[c, g, kk, 0] + n_row + KY[kk] - np.float32(1.0)
    px = off[c, g, kk, 1] + n_col + KX[kk] - np.float32(1.0)
    pcy = np.clip(py, -2.0, 49.0)
    pcx = np.clip(px, -2.0, 49.0)
    y0 = np.floor(pcy)
    fy = pcy - y0
    x0 = np.floor(pcx)
    fx = pcx - x0
    wy0 = (1.0 - fy) * (y0 == np.clip(y0, 0.0, 47.0))
    wy1 = fy * ((y0 + 1.0) == np.clip(y0 + 1.0, 0.0, 47.0))
    wx0 = (1.0 - fx) * (x0 == np.clip(x0, 0.0, 47.0))
    wx1 = fx * ((x0 + 1.0) == np.clip(x0 + 1.0, 0.0, 47.0))
    swap = (y0 == -1.0)
    top = np.where(swap, wy1, wy0)
    bot = np.where(swap, 0.0, wy1)
    y0c = np.clip(y0, 0.0, 47.0)
    jx = np.clip(x0 + 1.0, 0.0, 48.0)
    it = (y0c * 49.0 + jx).astype(np.int16)
    w4 = np.stack([top * wx0, top * wx1, bot * wx0, bot * wx1],
                  axis=-1).astype(np.float32)  # (N, 4)
    return it, w4


def host_inputs_for_core(I, b, half):
    r0 = HP * half
    rows = slice(r0, r0 + HP)
    out = {}
    out['qin'] = np.ascontiguousarray(
        np.asarray(I['q'])[b, 0, :, rows, :].reshape(C, N)).astype(np.float32)
    out['kin'] = np.ascontiguousarray(
        np.asarray(I['k'])[b].reshape(CLIP, C, H * W)).astype(np.float32)
    out['vin'] = np.ascontiguousarray(
        np.asarray(I['v'])[b].reshape(CLIP, C, H * W)).astype(np.float32)

    off = np.asarray(I['offset'])[b][:, :, rows, :].reshape(
        CLIP, DG, K, 2, N).astype(np.float32)
    n_row = ((np.arange(N) // W) + r0).astype(np.float32)
    n_col = (np.arange(N) % W).astype(np.float32)

    # cache quads per (c, g, kk)
    qc = {}
    for c in range(CLIP):
        for g in range(DG):
            for kk in range(K):
                qc[(c, g, kk)] = _quad(off, n_row, n_col, c, g, kk)

    # gather indices per (side, t) in gather-slot layout [128, 648]
    gx = np.zeros((6, 128, 648), np.int16)
    for si, side in enumerate(('k', 'v')):
        for t in range(3):
            for s in range(8):
                ch, hh = TILE_SLOTS[t][s]
                g = (hh // 2) + (6 if side == 'v' else 0)
                for kk in range(K):
                    it, _ = qc[(ch, g, kk)]
                    gx[si * 3 + t, 16 * s:16 * s + 16,
                       72 * kk:72 * (kk + 1)] = it.reshape(72, 16).T
    out['gidx'] = np.ascontiguousarray(gx)

    # k-side quad weights [108, N, 4]: row = kk*12 + c*6 + gl
    wqk = np.zeros((108, N, 4), np.float32)
    for c in range(CLIP):
        for gl in range(6):
            for kk in range(K):
                _, w4 = qc[(c, gl, kk)]
                wqk[kk * 12 + c * 6 + gl] = w4
    out['wqk'] = wqk.reshape(108, -1).astype(BF)

    # v-side quad weights per clip [108, N, 4]: row = kk*12 + h
    for c in range(CLIP):
        wqv = np.zeros((108, N, 4), np.float32)
        for hh in range(HEADS):
            for kk in range(K):
                _, w4 = qc[(c, 6 + hh // 2, kk)]
                wqv[kk * 12 + hh] = w4
        out[f'wqv{c}'] = wqv.reshape(108, -1).astype(BF)

    for nm, w in (('wqT', I['wq']), ('wkT', I['wk']), ('wvT', I['wv']),
                  ('w1T', I['w1']), ('w2T', I['w2'])):
        out[nm] = np.ascontiguousarray(np.asarray(w).T.astype(np.float32))
    for nm in ('bq', 'bk', 'bv', 'b1', 'b2'):
        out[nm] = np.asarray(I[nm]).astype(np.float32).reshape(-1, 1)

    blk32 = np.zeros((128, 32), np.float32)
    for s in range(8):
        blk32[16 * s:16 * s + 16, s] = 1.0
    out['blk32'] = blk32
    out['id128'] = np.eye(128, dtype=np.float32)
    zselh = np.zeros((108, 12), np.float32)
    for hh in range(HEADS):
        for kk in range(K):
            zselh[kk * 12 + hh, hh] = 1.0
    out['zselh'] = zselh
    sum2 = np.zeros((128, 64), np.float32)
    for j in range(64):
        sum2[j, j] = 1.0
        sum2[64 + j, j] = 1.0
    out['sum2'] = sum2
    ind8 = np.zeros((8, 128), np.float32)
    for s in range(8):
        ind8[s, 16 * s:16 * s + 16] = 1.0
    out['ind8'] = ind8
    return out


INPUT_SPECS = {
    'qin': ((C, N), F32), 'kin': ((CLIP, C, H * W), F32),
    'vin': ((CLIP, C, H * W), F32),
    'gidx': ((6, 128, 648), I16),
    'wqk': ((108, N * 4), BF16),
    'wqv0': ((108, N * 4), BF16), 'wqv1': ((108, N * 4), BF16),
    'wqT': ((C, C), F32), 'wkT': ((C, C), F32), 'wvT': ((C, C), F32),
    'w1T': ((C, 2 * C), F32), 'w2T': ((2 * C, C), F32),
    'bq': ((C, 1), F32), 'bk': ((C, 1), F32), 'bv': ((C, 1), F32),
    'b1': ((2 * C, 1), F32), 'b2': ((C, 1), F32),
    'blk32': ((128, 32), F32), 'id128': ((128, 128), F32),
    'zselh': ((108, 12), F32), 'sum2': ((128, 64), F32),
    'ind8': ((8, 128), F32),
}


# ======================================================================
# device program
# ======================================================================

@with_exitstack
def device_kernel(ctx: ExitStack, tc: tile.TileContext, outs, ins):
    nc = tc.nc
    out_dram = outs['out']

    pool = ctx.enter_context(tc.tile_pool(name="persist", bufs=1))
    pool3 = ctx.enter_context(tc.tile_pool(name="work", bufs=2))

    dma = nc.sync.dma_start
    dma2 = nc.scalar.dma_start
    dma3 = nc.sync.dma_start

    def loadt(name, eng=dma, pl=None, tag=None):
        shp, dt = INPUT_SPECS[name]
        src = ins[name]
        if len(shp) > 2:
            dims = " ".join(f"a{i}" for i in range(len(shp)))
            outer = " ".join(f"a{i}" for i in range(len(shp) - 1))
            src = src.rearrange(f"{dims} -> ({outer}) a{len(shp) - 1}")
        parts, fr = int(np.prod(shp[:-1])), shp[-1]
        tiles = []
        p0 = 0
        while p0 < parts:
            p = min(128, parts - p0)
            t = (pl or pool).tile([p, fr], dt, tag=(tag or name) + f"_{p0}",
                                  name=(tag or name) + f"_{p0}")
            eng(t[:], src[p0:p0 + p, :])
            tiles.append(t)
            p0 += p
        return tiles

    wq_t = loadt('wqT'); wk_t = loadt('wkT'); wv_t = loadt('wvT')
    w1_t = loadt('w1T'); w2_t = loadt('w2T')
    bq_t = loadt('bq'); bk_t = loadt('bk'); bv_t = loadt('bv')
    b1_t = loadt('b1'); b2_t = loadt('b2')
    sum2_t = loadt('sum2')[0]
    ind8_t = loadt('ind8')[0]

    def as_bf16(src_tiles, tag):
        t0 = src_tiles[0]
        t = pool.tile(list(t0[:].shape), BF16, tag=tag)
        nc.vector.tensor_copy(t[:], t0[:])
        return t

    qstack = ExitStack()
    qpp = qstack.enter_context(tc.tile_pool(name="qpersist", bufs=1))
    blk32_bf = as_bf16(loadt('blk32'), "blk32b")
    id_bf = as_bf16(loadt('id128'), "id128b")
    zselh_bf = as_bf16(loadt('zselh'), "zselhb")
    ind8_bf = as_bf16([ind8_t], "ind8b")

    # quad weights (bf16, host-computed)
    wqk_t = qpp.tile([108, N, 4], BF16, tag="wqk", name="wqk")
    dma(wqk_t[:].rearrange("p n l -> p (n l)"), ins['wqk'])
    wqv = {}

    # gather indices
    gidx = {}
    gsrc = ins['gidx'].rearrange("a b c -> (a b) c")
    for si, side in enumerate(('k', 'v')):
        for t in range(3):
            g = (qpp if side == 'k' else pool).tile(
                [128, 648], I16, tag=f"gx_{side}{t}")
            dma(g[:], gsrc[(si * 3 + t) * 128:(si * 3 + t + 1) * 128, :])
            gidx[(side, t)] = g

    kin, vin = ins['kin'], ins['vin']

    # ---------------- projections -> quad src tiles ----------------
    NCHW = 384

    def make_src():
        ts = [pool.tile([128, M3, 4], BF16, tag=f"src_{t}", name=f"src{t}")
              for t in range(3)]
        for t in ts:
            nc.vector.memset(t[:], 0.0)
        return ts

    def project_to_src(x_dram, w_tiles, b_tiles, src_tiles, psp, tmp_tag):
        for co0, co_p in ((0, 128), (128, 64)):
            for c in range(CLIP):
                for ncx in range(H // 8):
                    ps = psp.tile([co_p, NCHW], F32, tag="proj_ps")
                    xa = pool3.tile([128, NCHW], F32, tag="proj_in_a")
                    xb = pool3.tile([64, NCHW], F32, tag="proj_in_b")
                    dma(xa[:], x_dram[c, 0:128, ncx * NCHW:(ncx + 1) * NCHW])
                    dma(xb[:], x_dram[c, 128:192, ncx * NCHW:(ncx + 1) * NCHW])
                    nc.tensor.matmul(ps[:], w_tiles[0][:, co0:co0 + co_p],
                                     xa[:], start=True, stop=False)
                    nc.tensor.matmul(ps[:], w_tiles[1][:, co0:co0 + co_p],
                                     xb[:], start=False, stop=True)
                    bias = b_tiles[0 if co0 == 0 else 1]
                    y0 = 8 * ncx
                    psv = ps[:].rearrange("p (y x) -> p y x", y=8)
                    if co0 == 0 or c == 0:
                        dst = src_tiles[c] if co0 == 0 else src_tiles[2]
                        sl = dst[0:co_p, y0 * 49:(y0 + 8) * 49, :].rearrange(
                            "p (y x) l -> p y x l", y=8)
                        nc.scalar.activation(
                            sl[:, :, 0:48, 1], psv, AF.Identity, bias=bias[:])
                        nc.scalar.activation(
                            sl[:, :, 1:49, 0], psv, AF.Identity, bias=bias[:])
                    else:
                        tmp = pool3.tile([64, 8 * 49, 2], BF16,
                                         tag=f"t2tmp_{tmp_tag}")
                        nc.vector.memset(tmp[:], 0.0)
                        tv = tmp[:].rearrange("p (y x) l -> p y x l", y=8)
                        nc.scalar.activation(
                            tv[:, :, 0:48, 1], psv, AF.Identity, bias=bias[:])
                        nc.scalar.activation(
                            tv[:, :, 1:49, 0], psv, AF.Identity, bias=bias[:])
                        dma(src_tiles[2][64:128, y0 * 49:(y0 + 8) * 49, 0:2],
                            tmp[:])

    def fill_lane23(src_tiles):
        # lanes 2:4 of token m = lanes 0:2 of token m+49 (next image row)
        for t in src_tiles:
            nc.scalar.copy(t[:, 0:M3 - 49, 2:4], t[:, 49:M3, 0:2])

    # q projection -> q1 single-lane bf16
    q1 = {0: qpp.tile([128, N], BF16, tag="q1_0", name="q1a"),
          2: qpp.tile([128, N], BF16, tag="q1_2", name="q1b")}
    qp_tail = qpp.tile([64, N], BF16, tag="qp_tail")
    with tc.tile_pool(name="psq", bufs=2, space="PSUM") as psq, \
            tc.tile_pool(name="qpool", bufs=1) as qpl:
        qin_t = loadt('qin', pl=qpl)
        for co0, co_p in ((0, 128), (128, 64)):
            for ncx in range(3):
                nw = 384
                ps = psq.tile([co_p, nw], F32, tag="q_ps")
                nc.tensor.matmul(ps[:], wq_t[0][:, co0:co0 + co_p],
                                 qin_t[0][:, ncx * nw:(ncx + 1) * nw],
                                 start=True, stop=False)
                nc.tensor.matmul(ps[:], wq_t[1][:, co0:co0 + co_p],
                                 qin_t[1][:, ncx * nw:(ncx + 1) * nw],
                                 start=False, stop=True)
                if co0 == 0:
                    nc.vector.tensor_scalar(
                        q1[0][:, ncx * nw:(ncx + 1) * nw],
                        ps[:], bq_t[0][:], SCALE, ALU.add, ALU.mult)
                else:
                    nc.vector.tensor_scalar(
                        qp_tail[:, ncx * nw:(ncx + 1) * nw],
                        ps[:], bq_t[1][:], SCALE, ALU.add, ALU.mult)
        dma(q1[2][0:64, :], qp_tail[:])
        dma(q1[2][64:128, :], qp_tail[:])

    q4 = {}
    for tkey in (0, 2):
        q4[tkey] = qpp.tile([128, N, 4], BF16, tag=f"q4_{tkey}",
                            name=f"q4t{tkey}")
        for lane in range(4):
            nc.scalar.copy(q4[tkey][:, :, lane], q1[tkey][:])

    pspj = ctx.enter_context(
        tc.tile_pool(name="pspj", bufs=1, space="PSUM"))
    ksrc = make_src()
    project_to_src(kin, wk_t, bk_t, ksrc, pspj, 'k')
    fill_lane23(ksrc)

    # ---------------- k-wave ----------------
    attn = [pool.tile([108, N], BF16, tag=f"attn_{c}", name=f"attn{c}")
            for c in range(CLIP)]

    with tc.tile_pool(name="pskw", bufs=1, space="PSUM") as pskw, \
            tc.tile_pool(name="kwork", bufs=2) as kw, \
            tc.tile_pool(name="kp", bufs=1) as kp:
        for t in range(3):
            for kg in range(3):
                kks = [3 * kg, 3 * kg + 1, 3 * kg + 2]
                # quad weight rows in psum-row layout [96, N, 4]
                wr = kw.tile([96, N, 4], BF16, tag="kg_wr", bufs=1)
                nc.vector.memset(wr[:], 0.0)
                for ki, kk in enumerate(kks):
                    if t < 2:
                        c0 = TILE_SLOTS[t][0][0]
                        base = kk * 12 + c0 * 6
                        src = wqk_t[base:base + 4, :, :].unsqueeze(
                            1).broadcast_to([4, 2, N, 4])
                        dma2(wr[32 * ki:32 * ki + 8, :, :], src)
                    else:
                        for ch in range(2):
                            base = kk * 12 + ch * 6 + 4
                            src = wqk_t[base:base + 2, :, :].unsqueeze(
                                1).broadcast_to([2, 2, N, 4])
                            dma2(wr[32 * ki + 4 * ch:
                                    32 * ki + 4 * ch + 4, :, :], src)
                # gathers + q-mul into separate p tiles (frees g fast)
                ps_list = []
                for ki, kk in enumerate(kks):
                    g = kp.tile([128, N, 4], BF16, tag="kg_g", bufs=2)
                    nc.gpsimd.ap_gather(
                        g[:], ksrc[t][:],
                        gidx[('k', t)][:, 72 * kk:72 * (kk + 1)],
                        channels=128, num_elems=M3, d=4, num_idxs=N)
                    qt = q4[0 if t < 2 else 2]
                    p = kp.tile([128, N, 4], BF16, tag=f"kg_p{ki}")
                    nc.vector.tensor_mul(p[:], g[:], qt[:])
                    ps_list.append(p)
                sc = kw.tile([96, N], F32, tag="kg_sc", bufs=1)
                for half in range(2):
                    h0 = half * 2304
                    n0 = half * 576
                    ps = pskw.tile([96, 2304], F32, tag="dots")
                    for ki in range(3):
                        pr = ps_list[ki][:].rearrange("p n l -> p (n l)")
                        for c0 in range(0, 2304, 512):
                            cw = min(512, 2304 - c0)
                            nc.tensor.matmul(
                                ps[32 * ki:32 * ki + 32, c0:c0 + cw],
                                blk32_bf[:], pr[:, h0 + c0:h0 + c0 + cw],
                                start=True, stop=True,
                                tile_position=(0, 32 * ki))
                    e = kw.tile([96, 2304], F32, tag="kg_e", bufs=1)
                    wrf = wr[:].rearrange("p n l -> p (n l)")
                    nc.any.tensor_tensor(e[:], ps[:], wrf[:, h0:h0 + 2304],
                                         ALU.mult)
                    ev = e[:].rearrange("p (n l) -> p n l", l=4)
                    s1 = kw.tile([96, 576, 2], F32, tag="kg_s1", bufs=1)
                    nc.vector.tensor_add(s1[:], ev[:, :, 0:2], ev[:, :, 2:4])
                    nc.vector.tensor_add(sc[:, n0:n0 + 576],
                                         s1[:, :, 0], s1[:, :, 1])
                esc = kw.tile([96, N], BF16, tag="kg_esc", bufs=1)
                nc.scalar.activation(esc[:], sc[:], AF.Exp)
                # attn scatter
                for ki, kk in enumerate(kks):
                    if t < 2:
                        c0 = TILE_SLOTS[t][0][0]
                        dma3(attn[c0][kk * 12:kk * 12 + 8, :],
                             esc[32 * ki:32 * ki + 8, :])
                    else:
                        for ch in range(2):
                            dma3(attn[ch][kk * 12 + 8:kk * 12 + 12, :],
                                 esc[32 * ki + 4 * ch:
                                     32 * ki + 4 * ch + 4, :])

    # ---------------- Z from attn tiles (PE) ----------------
    qstack.close()
    vpp = ctx.enter_context(tc.tile_pool(name="vpersist", bufs=1))
    for c in range(CLIP):
        wqv[c] = vpp.tile([108, N, 4], BF16, tag=f"wqv{c}", name=f"wqv{c}")
        dma(wqv[c][:].rearrange("p n l -> p (n l)"), ins[f'wqv{c}'])
    zpool = ctx.enter_context(tc.tile_pool(name="zpool", bufs=1))
    zsum = zpool.tile([HEADS, N], F32, tag="zsum")
    with tc.tile_pool(name="psz", bufs=1, space="PSUM") as psz:
        zp = psz.tile([HEADS, N], F32, tag="zp")
        for ci in range(CLIP):
            for c0 in range(0, N, 512):
                cw = min(512, N - c0)
                nc.tensor.matmul(zp[:, c0:c0 + cw], zselh_bf[:],
                                 attn[ci][:, c0:c0 + cw],
                                 start=(ci == 0), stop=(ci == 1))
        nc.vector.reciprocal(zsum[:], zp[:])

    # ---------------- v-wave ----------------
    vsrc = make_src()
    project_to_src(vin, wv_t, bv_t, vsrc, pspj, 'v')
    fill_lane23(vsrc)

    # awq[c] = wqv[c] * attn[c] (per-lane)
    for c in range(CLIP):
        for lane in range(4):
            nc.vector.tensor_mul(wqv[c][:, :, lane], attn[c][:],
                                 wqv[c][:, :, lane])

    # zr replication via PE outer product (fp32)
    tail = ctx.enter_context(tc.tile_pool(name="tail", bufs=1))
    zr_main = tail.tile([128, N], F32, tag="zr_main")
    zr_tail = tail.tile([64, N], F32, tag="zr_tail")
    with tc.tile_pool(name="pszr", bufs=1, space="PSUM") as pszr:
        zpm = pszr.tile([128, N], F32, tag="zr_ps")
        for c0 in range(0, N, 512):
            cw = min(512, N - c0)
            nc.tensor.matmul(zpm[:, c0:c0 + cw], ind8_t[:],
                             zsum[0:8, c0:c0 + cw], start=True, stop=True)
        nc.vector.tensor_copy(zr_main[:], zpm[:])
        ztail4 = zpool.tile([4, N], F32, tag="ztail4")
        dma(ztail4[:], zsum[8:12, :])
        zpt = pszr.tile([64, N], F32, tag="zrt_ps")
        for c0 in range(0, N, 512):
            cw = min(512, N - c0)
            nc.tensor.matmul(zpt[:, c0:c0 + cw], ind8_t[0:4, 0:64],
                             ztail4[:, c0:c0 + cw], start=True, stop=True)
        nc.vector.tensor_copy(zr_tail[:], zpt[:])

    o_main = tail.tile([128, N], F32, tag="o_main")
    t2sb = tail.tile([128, N], F32, tag="t2sb")
    with tc.tile_pool(name="psacc", bufs=1, space="PSUM") as psacc, \
            tc.tile_pool(name="vwork", bufs=2) as vw:
        acc_main = psacc.tile([128, N], F32, tag="acc_main")
        acc_t2 = psacc.tile([128, N], F32, tag="acc_t2")
        nmm = [0, 0]
        for t in range(3):
            for kk in range(K):
                g = vw.tile([128, N, 4], BF16, tag="vg_g")
                nc.gpsimd.ap_gather(
                    g[:], vsrc[t][:],
                    gidx[('v', t)][:, 72 * kk:72 * (kk + 1)],
                    channels=128, num_elems=M3, d=4, num_idxs=N)
                # wr via PE outer-product replication of awq rows
                wrows = vw.tile([8, N, 4], BF16, tag="vg_wrows", bufs=1)
                if t < 2:
                    c0 = TILE_SLOTS[t][0][0]
                    dma(wrows[:], wqv[c0][kk * 12:kk * 12 + 8, :, :])
                else:
                    for ch in range(2):
                        dma(wrows[4 * ch:4 * ch + 4, :, :],
                            wqv[ch][kk * 12 + 8:kk * 12 + 12, :, :])
                wrowf = wrows[:].rearrange("p n l -> p (n l)")
                wrsb = vw.tile([128, N, 4], BF16, tag="vg_wr", bufs=1)
                wrsbf = wrsb[:].rearrange("p n l -> p (n l)")
                for q in range(9):
                    q0 = q * 512
                    wrps = psacc.tile([128, 512], F32, tag="vg_wrps")
                    nc.tensor.matmul(wrps[:], ind8_bf[:],
                                     wrowf[:, q0:q0 + 512],
                                     start=True, stop=True)
                    nc.scalar.copy(wrsbf[:, q0:q0 + 512], wrps[:])
                nc.vector.tensor_mul(g[:], g[:], wrsb[:])
                nc.vector.tensor_add(g[:, :, 0:2], g[:, :, 0:2],
                                     g[:, :, 2:4])
                ms = vw.tile([128, N], BF16, tag="vg_ms", bufs=1)
                nc.vector.tensor_add(ms[:], g[:, :, 0], g[:, :, 1])
                zi = 0 if t < 2 else 1
                accp = acc_main if t < 2 else acc_t2
                nmm[zi] += 1
                last = (nmm[zi] == (2 * K if zi == 0 else K))
                for c0 in range(0, N, 512):
                    cw = min(512, N - c0)
                    nc.tensor.matmul(accp[:, c0:c0 + cw], id_bf[:],
                                     ms[:, c0:c0 + cw],
                                     start=(nmm[zi] == 1), stop=last)
        nc.vector.tensor_mul(o_main[:], acc_main[:], zr_main[:])
        nc.vector.tensor_copy(t2sb[:], acc_t2[:])

    # ---------------- tail sum + MLP ----------------
    with tc.tile_pool(name="psmlp", bufs=2, space="PSUM") as psm:
        o_tail = tail.tile([64, N], F32, tag="o_tail")
        for c0 in range(0, N, 384):
            ps = psm.tile([64, 384], F32, tag="t2_ps")
            nc.tensor.matmul(ps[:], sum2_t[:, 0:64], t2sb[:, c0:c0 + 384],
                             start=True, stop=True)
            nc.vector.tensor_mul(o_tail[:, c0:c0 + 384], ps[:],
                                 zr_tail[:, c0:c0 + 384])

        h1 = [tail.tile([128, N], F32, tag=f"h1_{i}", name=f"h1{i}")
              for i in range(3)]
        for i in range(3):
            for ncx in range(3):
                nw = 384
                ps = psm.tile([128, nw], F32, tag="mlp_ps")
                nc.tensor.matmul(ps[:], w1_t[0][:, 128 * i:128 * (i + 1)],
                                 o_main[:, ncx * nw:(ncx + 1) * nw],
                                 start=True, stop=False)
                nc.tensor.matmul(ps[:], w1_t[1][:, 128 * i:128 * (i + 1)],
                                 o_tail[:, ncx * nw:(ncx + 1) * nw],
                                 start=False, stop=True)
                nc.scalar.activation(h1[i][:, ncx * nw:(ncx + 1) * nw],
                                     ps[:], AF.Gelu, bias=b1_t[i][:])
        out_sb = [tail.tile([128, N], F32, tag="out0", name="outsb0"),
                  tail.tile([64, N], F32, tag="out1", name="outsb1")]
        for i, (co0, co_p) in enumerate(((0, 128), (128, 64))):
            for ncx in range(3):
                nw = 384
                ps = psm.tile([co_p, nw], F32, tag="mlp_ps2")
                for j in range(3):
                    nc.tensor.matmul(ps[:], w2_t[j][:, co0:co0 + co_p],
                                     h1[j][:, ncx * nw:(ncx + 1) * nw],
                                     start=(j == 0), stop=(j == 2))
                osrc = o_main if i == 0 else o_tail
                bias = b2_t[0] if i == 0 else b2_t[1]
                tmp = pool3.tile([co_p, nw], F32, tag="mlp_tmp")
                nc.vector.tensor_scalar(tmp[:], ps[:], bias[:], None, ALU.add)
                nc.vector.tensor_add(out_sb[i][:, ncx * nw:(ncx + 1) * nw],
                                     tmp[:], osrc[:, ncx * nw:(ncx + 1) * nw])
    dma(out_dram[0:128, :], out_sb[0][:])
    dma(out_dram[128:192, :], out_sb[1][:])


# ======================================================================
# launch
# ======================================================================

def _build_program():
    import concourse.bacc as bacc
    nc = bacc.Bacc("TRN2", target_bir_lowering=False, debug=False,
                   num_devices=8)
    in_aps = {}
    for name, (shp, dt) in INPUT_SPECS.items():
        in_aps[name] = nc.dram_tensor(
            name, list(shp), dt, kind="ExternalInput").ap()
    out_ap = nc.dram_tensor("out", [C, N], F32, kind="ExternalOutput").ap()
    with tile.TileContext(nc) as t:
        device_kernel(t, {'out': out_ap}, in_aps)
    nc.compile()
    return nc


_PROGRAM = None


def kernel(**inputs):
    global _PROGRAM
    from concourse import bass_utils
    if _PROGRAM is None:
        _PROGRAM = _build_program()
    in_maps = []
    for core in range(8):
        b, half = core // 2, core % 2
        in_maps.append(host_inputs_for_core(inputs, b, half))
    res = bass_utils.run_bass_kernel_spmd(
        _PROGRAM, in_maps, core_ids=list(range(8)))
    out = np.zeros((4, 1, C, H, W), np.float32)
    for core in range(8):
        b, half = core // 2, core % 2
        o = res.results[core]['out'].reshape(C, HP, W)
        out[b, 0, :, HP * half:HP * (half + 1), :] = o
    return out
